# revision 25
# baseline (speedup 1.0000x reference)
"""Trainium2 Bass kernel for nn_AttentionModule (B=8, C=128, H=W=256).

out[b,c] = softmax((W1 x_b + b1)[c] @ ((W2 x_b + b2)[c])^T) @ (W2 x_b + b2)[c] + x_b[c]

Sharding: data-parallel over batch B across the 8 NeuronCores (1 batch each);
weights replicated. Each core runs an identical single-core NEFF.

Datapath: fp16 (fp32 matmuls are 4 cyc/col on TRN2, 16-bit 1 cyc/col; fp16's
10-bit mantissa keeps scores accurate enough, absmax rel err ~7e-3 vs 2e-2
gate). P is bf16 (unnormalized exp spans ~e^-150..e^21, too wide for fp16).

  Phase A (per 64-channel group, x streamed in fp16):
    trick-GEMM per (h, w-chunk): stationary lhsT = x[:, h, wchunk] fp16,
    moving rhs = [W1^T|W2^T] group columns (N=128) -> PSUM [w,128] holds
    q^T|k^T for this h directly in [w, ...] layout. qT/kT free layout is
    (h, wc, c) so the PSUM->SBUF evacuation writes one contiguous 256-run
    (an (wc,c,h) layout makes the evac write 2-elem scattered runs, ~5x
    slower on DVE/ACT). Evac: q on DVE (tensor_add with b1 pattern,
    fp32 PSUM -> fp16 SBUF), k on ACT (copy). Matmul operands then read
    qT/kT with stride-128 APs (PE streams strided at full rate).
  Phase B, channels in batches of 2 (a short 8-transpose burst, then a
  pure-matmul stretch -- transpose-mode is invisible to the HAM clock
  gate, so bursts are kept well under its 3.4us window):
    sT[g,h] = sum_w kT[w,g] qT[w,h]   (k chunks stationary; 4 MM N=256)
    PT = exp(sT - SHIFT) on ACT, PSUM -> SBUF bf16 directly (this IS the
    "transposed P" needed by the out matmul; constant shift instead of a
    row max -- scores are bounded well inside exp/fp32 range).
    kn[g,w] = PE-transpose of kT chunks (fp16, 4x 128-col transposes),
    evac on DVE with baked +b2[c] immediate; col 256 of kn is a constant 1
    (16 fixed kn buffers, ones written once).
    po[h, 0:257] = PT^T @ [kn | ones]: cols 0:256 = unnormalized out,
    col 256 = l[h] = sum_g exp(...) (softmax denominator, lands
    per-partition for free). rinv = 1/l on DVE.
    ob(ht0) = po*rinv + x  (DVE scalar_tensor_tensor); ob(ht1) split as
    ACT copy-with-scale then a GPSIMD tensor_add for engine balance.
    The +b2 in kn makes this exactly out+b2+x (softmax rows sum to 1);
    b1's score contribution is the b1 pattern added to q in Phase A.
  PSUM (8 banks): ring of 4 [128,512] slots shared by Phase-A tiles and
  sT (2-channel slack on every PE wait), knps x2, po x2.
  A ~24-matmul warmup burst at start flips the PE clock gate to 2.4 GHz
  through Phase A (Phase B's per-channel sem waits keep re-throttling it;
  measured ~85% of Phase B runs at K=4/8 1.2 GHz -- see HAM notes in the
  tensor-engine doc).
  I/O: x in/out as fp16 (host converts), residual x re-read per 4-channel
  batch as one 512KB DMA; out written as one 512KB DMA per 4 channels.

Bias algebra: k is kept UNBIASED on chip. b2 shifts scores by a per-row
constant (softmax-invariant) and contributes exactly +b2[c] to the output
(rows sum to 1) -- folded into kn's evac. b1 is added to q.

Container workarounds (see _apply_tile_patches):
  - walrus here encodes at most one sem wait per instruction -> split.
  - EVSEM butterfly barrier hangs at runtime -> NRT pseudo barrier.
  - sem_clear/dma_reset hang -> skipped (one execution per model load).
  - HWDGE (nc.sync) DMAs hang under Tile -> all DMAs on gpsimd (SWDGE).
"""

import sys

if '/opt/trn_rl_repo' not in sys.path:
    sys.path.insert(0, '/opt/trn_rl_repo')

import numpy as np

B, C, H, W = 8, 128, 256, 256
CG = 64           # channels per group
NCG = C // CG     # 2 groups / x passes
HSLAB = 8         # h rows per x DMA slab (512KB)
HSUB = 2          # h rows per Phase-A PSUM bank step
CB = 4            # channels per xr/out DMA batch
N_CORES = 8
HW_ELEMS = H * W
SHIFT = 75.0      # softmax constant shift (max |score| ~ 96 << SHIFT+87)

_patched = False


def _apply_tile_patches():
    global _patched
    if _patched:
        return
    _patched = True
    import concourse.tile as tile
    from concourse.vector_clock import ScopedClock

    def _drain_and_barrier(self, tick_clock, wait_clock):
        nc = self.nc
        drain_inst = nc.sync.drain()
        wait_clock.add_sem_waits(
            drain_inst.ins, ScopedClock({None: tick_clock.global_clock})
        )
        nc._nrt_pseudo_barrier()
        assert self.sems is not None
        popped = nc._tile_sem_poison_stack.pop()
        assert popped is self._sem_poison
        # No sem_clear / dma_reset: RANGE_CLEAR and DMA_RESET hang on this
        # runtime. Sound because every kernel() call loads a fresh
        # executable (NRT zeroes semaphores at load).

    tile.TileContext._drain_and_barrier = _drain_and_barrier


def _split_multi_waits(nc):
    from concourse import mybir
    n = 0
    for f in nc.m.functions:
        for blk in f.blocks:
            insts = list(blk.instructions)
            out = []
            changed = False
            for inst in insts:
                si = getattr(inst, "sync_info", None)
                if si is not None and len(si.on_wait) > 1:
                    waits = list(si.on_wait)
                    for i, w in enumerate(waits[:-1]):
                        nop = mybir.InstNoOp(
                            name=f"{inst.name}_wsplit{i}", ins=[], outs=[])
                        nop.engine = inst.engine
                        nop.sync_info = mybir.SyncInfo(on_wait=[w], on_update=[])
                        out.append(nop)
                        n += 1
                    inst.sync_info = mybir.SyncInfo(
                        on_wait=[waits[-1]], on_update=list(si.on_update))
                    changed = True
                out.append(inst)
            if changed:
                blk.instructions = out
    return n


def build_program(b2, patch=True):
    """Build the single-core Bass program. b2 values are baked as
    immediates into the kn evacuation ops. Returns nc."""
    if patch:
        _apply_tile_patches()
    import concourse.bass as bass
    import concourse.tile as tile
    from concourse import mybir
    from contextlib import ExitStack

    f32 = mybir.dt.float32
    bf16 = mybir.dt.bfloat16
    f16 = mybir.dt.float16
    AF = mybir.ActivationFunctionType
    ALU = mybir.AluOpType

    b2 = [float(v) for v in b2]
    assert len(b2) == C

    nc = bass.Bass("TRN2", target_bir_lowering=False, debug=False, num_devices=1)
    x_t = nc.dram_tensor("x", [C, H, W], f16, kind="ExternalInput")
    wcat_t = nc.dram_tensor("wcat", [C, NCG * 128], f16, kind="ExternalInput")
    biasq_t = nc.dram_tensor("biasq", [128, NCG * HSUB * 128], f32, kind="ExternalInput")
    ident_t = nc.dram_tensor("ident", [128, 128], f16, kind="ExternalInput")
    out_t = nc.dram_tensor("out", [C, H, W], f16, kind="ExternalOutput")

    x_ap = x_t.ap()       # [128(c), 256, 256]
    QKW = 2 * CG * H      # free size of qT/kT group tiles (wc, c, h)

    def dram_cb_slab(tensor, c0):
        # [h%128 (partitions), cb(CB), ht(2), w] view of [C,H,W] for
        # channels c0..c0+CB  (each contiguous run = one 512B w-row)
        return bass.AP(tensor.ap().tensor, c0 * HW_ELEMS,
                       [[W, 128], [HW_ELEMS, CB], [128 * W, 2], [1, W]])

    with tile.TileContext(nc) as tc, ExitStack() as ctx:
        consts = ctx.enter_context(tc.tile_pool(name="consts", bufs=1))
        gq = ctx.enter_context(tc.tile_pool(name="gq", bufs=1))
        gk = ctx.enter_context(tc.tile_pool(name="gk", bufs=1))
        xpool = ctx.enter_context(tc.tile_pool(name="xpool", bufs=3))
        ptpool = ctx.enter_context(tc.tile_pool(name="ptpool", bufs=3))
        knpool = ctx.enter_context(tc.tile_pool(name="knpool", bufs=1))
        xrpool = ctx.enter_context(tc.tile_pool(name="xrpool", bufs=2))
        opool = ctx.enter_context(tc.tile_pool(name="opool", bufs=2))
        stats = ctx.enter_context(tc.tile_pool(name="stats", bufs=4))
        psX = ctx.enter_context(tc.tile_pool(name="psX", bufs=4, space="PSUM"))
        psK = ctx.enter_context(tc.tile_pool(name="psK", bufs=2, space="PSUM"))
        psO = ctx.enter_context(tc.tile_pool(name="psO", bufs=2, space="PSUM"))

        wcat_sb = consts.tile([128, NCG * 128], f16)
        nc.gpsimd.dma_start(out=wcat_sb[:], in_=wcat_t.ap())
        ident_sb = consts.tile([128, 128], f16)
        nc.gpsimd.dma_start(out=ident_sb[:], in_=ident_t.ap())
        biasq_sb = consts.tile([128, NCG * HSUB * 128], f32)
        nc.gpsimd.dma_start(out=biasq_sb[:], in_=biasq_t.ap())
        shift_sb = consts.tile([128, 1], f32)
        nc.vector.memset(shift_sb[:], -SHIFT)

        # HAM warmup: ~12 back-to-back dummy matmuls (5.1us at the cold
        # clock, past the ~3.4us flip window) flip the PE clock gate
        # to K=8/8 (2.4 GHz) before real work; it stays warm because no
        # >=3.4us contiguous PE-idle window exists later in the kernel.
        warm_sb = consts.tile([128, 512], f16)
        nc.vector.memset(warm_sb[:], 0.0)
        for wi in range(12):
            wps = psX.tile([128, 512], f32, tag="ps")
            nc.tensor.matmul(out=wps[:], lhsT=warm_sb[:, 0:128],
                             rhs=warm_sb[:], start=True, stop=True)

        # 4 fixed kn buffers [gc(2) x 257]: cols 256/513 are constant 1.0
        # (written once; evacs only touch the 256 data cols of each half)
        kn_bufs = []
        for i in range(8):
            knb = knpool.tile([128, 2 * 257], bf16, tag=f"kn{i}")
            ones_ap = bass.AP(knb[:].tensor, knb[:].offset + 256, [knb[:].ap[0], [257, 2], [1, 1]])
            nc.vector.memset(ones_ap, 1.0)
            kn_bufs.append(knb)

        for g in range(NCG):
            # group-resident qT/kT: [128(w), wc(2) x c(CG) x h(H)] bf16
            qT = gq.tile([128, QKW], f16, tag="qT")
            kT = gk.tile([128, QKW], f16, tag="kT")

            # ---------------- Phase A ----------------
            for hb in range(0, H, HSLAB):
                xt = xpool.tile([128, HSLAB * W], f16, tag="xt")
                nc.gpsimd.dma_start(
                    out=xt[:].rearrange("p (a b) -> p a b", a=HSLAB),
                    in_=x_ap[:, hb:hb + HSLAB, :])
                for sub in range(HSLAB // HSUB):
                    # one PSUM bank [128, 512]: (i(HSUB), wc(2), [q64|k64])
                    ps = psX.tile([128, 512], f32, tag="ps")
                    for i in range(HSUB):
                        for wc in range(2):
                            col = i * 256 + wc * 128
                            nc.tensor.matmul(
                                out=ps[:, col: col + 128],
                                lhsT=xt[:, (sub * HSUB + i) * W + wc * 128:
                                        (sub * HSUB + i) * W + wc * 128 + 128],
                                rhs=wcat_sb[:, g * 128:(g + 1) * 128],
                                start=(col == 0),
                                stop=(col == 384),
                            )
                    h0 = hb + sub * HSUB
                    # in dims (i, wc, c): strides (256, 128, 1)
                    ps_q = bass.AP(ps[:].tensor, ps[:].offset,
                                   [ps[:].ap[0], [256, HSUB], [128, 2], [1, CG]])
                    ps_k = bass.AP(ps[:].tensor, ps[:].offset + CG,
                                   [ps[:].ap[0], [256, HSUB], [128, 2], [1, CG]])
                    bq = bass.AP(biasq_sb[:].tensor,
                                 biasq_sb[:].offset + g * HSUB * 128,
                                 [biasq_sb[:].ap[0], [128, HSUB], [64, 2], [1, CG]])
                    # out dims (i, wc, c): layout (h, wc, c) -> strides
                    # (128, 64, 1): dst collapses to one contiguous 256-run
                    q_out = bass.AP(qT[:].tensor, qT[:].offset + h0 * 128,
                                    [qT[:].ap[0], [128, HSUB], [CG, 2], [1, CG]])
                    k_out = bass.AP(kT[:].tensor, kT[:].offset + h0 * 128,
                                    [kT[:].ap[0], [128, HSUB], [CG, 2], [1, CG]])
                    nc.vector.tensor_add(q_out, ps_q, bq)
                    nc.scalar.activation(k_out, ps_k, AF.Copy)

            # ---------------- Phase B (software-pipelined over channels:
            # stage1(c+1) [scores^T + exp + kn] is emitted before stage2(c)
            # [out matmul + normalize] so PE keeps streaming) -------------
            def kslice(wc, cl, gc):
                # kT chunk [w(128), g(128)] for (wc, channel, g-chunk):
                # element g=gc*128+j at free offset (gc*128+j)*128 + wc*64 + c
                return bass.AP(kT[:].tensor,
                               kT[:].offset + gc * 128 * 128 + wc * CG + cl,
                               [kT[:].ap[0], [128, 128]])

            def kn_stage(cl):
                # transpose burst for one channel (HAM-invisible; kept short
                # by batching only 2 channels between matmul stretches)
                knps = psK.tile([128, 512], f16, tag="knps")
                for gc in range(2):
                    for wc in range(2):
                        col = gc * 256 + wc * 128
                        nc.tensor.matmul(
                            out=knps[:, col: col + 128],
                            lhsT=kslice(wc, cl, gc),
                            rhs=ident_sb[:], is_transpose=True,
                            start=(gc == 0 and wc == 0),
                            stop=(gc == 1 and wc == 1),
                        )
                kn = kn_bufs[cl % 8]
                kn_data = bass.AP(kn[:].tensor, kn[:].offset,
                                  [kn[:].ap[0], [257, 2], [1, 256]])
                knps_in = bass.AP(knps[:].tensor, knps[:].offset,
                                  [knps[:].ap[0], [256, 2], [1, 256]])
                nc.vector.tensor_scalar_add(kn_data, knps_in, b2[g * CG + cl])
                return kn

            def st_stage(cl):
                sT = psX.tile([128, 512], f32, tag="ps")
                for gc in range(2):
                    for wc in range(2):
                        nc.tensor.matmul(
                            out=sT[:, gc * 256: gc * 256 + 256],
                            lhsT=kslice(wc, cl, gc),
                            rhs=bass.AP(qT[:].tensor, qT[:].offset + wc * CG + cl,
                                        [qT[:].ap[0], [128, 256]]),
                            start=(gc == 0 and wc == 0),
                            stop=(gc == 1 and wc == 1),
                        )
                pt = ptpool.tile([128, 512], bf16, tag="pt")
                nc.scalar.activation(pt[:], sT[:], AF.Exp,
                                     bias=shift_sb[:], scale=1.0)
                return pt

            def stage2(cl, pt, kn, xr, ob, bi):
                rinv = stats.tile([128, 2], f32, tag="rinv")
                pos = []
                for ht in range(2):
                    po = psO.tile([128, 512], f32, tag="po")
                    for gc in range(2):
                        nc.tensor.matmul(
                            out=po[:, 0:257],
                            lhsT=pt[:, gc * 256 + ht * 128: gc * 256 + ht * 128 + 128],
                            rhs=kn[:, gc * 257: gc * 257 + 257],
                            start=(gc == 0), stop=(gc == 1),
                        )
                    nc.vector.reciprocal(rinv[:, ht:ht + 1], po[:, 256:257])
                    pos.append(po)
                off0 = bi * 2 * 256
                nc.vector.scalar_tensor_tensor(
                    out=ob[:, off0:off0 + 256], in0=pos[0][:, 0:256],
                    scalar=rinv[:, 0:1], in1=xr[:, off0:off0 + 256],
                    op0=ALU.mult, op1=ALU.add)
                off1 = off0 + 256
                onrm = stats.tile([128, 256], f16, tag="onrm")
                nc.scalar.activation(onrm[:], pos[1][:, 0:256], AF.Copy,
                                     scale=rinv[:, 1:2])
                nc.gpsimd.tensor_add(ob[:, off1:off1 + 256], onrm[:],
                                     xr[:, off1:off1 + 256])

            xr = None
            ob = None
            pend = []      # stage2 args awaiting emission (previous batch)
            for cb0 in range(0, CG, 4):
                batch = []
                for cl in range(cb0, cb0 + 4):
                    bi = cl % CB
                    if bi == 0:
                        xr = xrpool.tile([128, CB * 512], f16, tag="xr")
                        nc.gpsimd.dma_start(out=xr[:],
                                            in_=dram_cb_slab(x_t, g * CG + cl))
                        ob = opool.tile([128, CB * 512], f16, tag="ob")
                    batch.append((cl, xr, ob, bi))
                # transpose burst (short: 8 transposes)
                kns = {cl: kn_stage(cl) for cl, _, _, _ in batch}
                # matmul stretch: sT of this batch interleaved with the
                # previous batch's out-matmuls
                newpend = []
                for j, (cl, xrj, obj, bij) in enumerate(batch):
                    pt = st_stage(cl)
                    newpend.append((cl, pt, kns[cl], xrj, obj, bij))
                    if j < len(pend):
                        stage2(*pend[j])
                        pcl, pob = pend[j][0], pend[j][4]
                        if pcl % CB == CB - 1:
                            nc.gpsimd.dma_start(
                                out=dram_cb_slab(out_t, g * CG + pcl - (CB - 1)),
                                in_=pob[:])
                pend = newpend
            for args in pend:
                stage2(*args)
                pcl, pob = args[0], args[4]
                if pcl % CB == CB - 1:
                    nc.gpsimd.dma_start(
                        out=dram_cb_slab(out_t, g * CG + pcl - (CB - 1)),
                        in_=pob[:])
    return nc


def _host_inputs(x_b, W1, b1, W2, b2):
    wcat = np.empty((C, NCG * 128), np.float32)
    for g in range(NCG):
        for cl in range(CG):
            wcat[:, g * 128 + cl] = W1[g * CG + cl, :]
            wcat[:, g * 128 + 64 + cl] = W2[g * CG + cl, :]
    biasq = np.empty((128, NCG * HSUB * 128), np.float32)
    for g in range(NCG):
        pat = np.empty((HSUB, 2, CG), np.float32)
        pat[:, :, :] = b1[g * CG:(g + 1) * CG][None, None, :]
        biasq[:, g * HSUB * 128:(g + 1) * HSUB * 128] = pat.reshape(-1)[None, :]
    ident = np.eye(128, dtype=np.float32)
    return {"x": np.ascontiguousarray(x_b).astype(np.float16),
            "wcat": wcat.astype(np.float16),
            "biasq": biasq, "ident": ident.astype(np.float16)}


def kernel(x, W1, b1, W2, b2, _trace=False):
    import concourse.bass_utils as bass_utils

    nc = build_program(np.asarray(b2, np.float64), patch=True)
    _split_multi_waits(nc)

    in_maps = [_host_inputs(np.asarray(x)[b], W1, b1, W2, b2) for b in range(B)]
    kw = {}
    if _trace:
        kw = dict(trace=True, trace_cores=[0])
    res = bass_utils.run_bass_kernel_spmd(
        nc, in_maps, core_ids=list(range(N_CORES)), **kw)
    out = np.stack([np.asarray(res.results[b]["out"], np.float32)
                    for b in range(B)], axis=0)
    if _trace:
        kernel._last_results = res
    return out
